# revision 1
# baseline (speedup 1.0000x reference)
"""Multi-head attention (B=4, S=2048, D=1024, H=16, causal) on 8 TRN2 cores.

Sharding: data-parallel over batch (4) x tensor-parallel over heads (2 groups
of 8). Core c handles batch c//2, head group c%2. Each core computes a partial
output projection (its 8 heads through its Wo rows); the host sums the two
partials per batch.

Device algorithm (per core, bf16 matmul operands, fp32 psum accumulation):
  - Host supplies transposed bf16 activations xT/kvT [D, S] and transposed
    bf16 weight shards (q-scale 1/sqrt(hd) folded into Wq).
  - kT [e, sk] and v [sk, e] from projection matmuls; qT per sq block.
  - Scores TRANSPOSED: ST[sk, sq] = kT_h.T @ qT_h per 128-wide sk tile, with
    the moving width trimmed at the causal diagonal. exp on ACT into bf16
    pexp tiles; exact-diagonal 128x128 subtile masked by GPSIMD affine_select.
  - ctx in [sq, hd] orientation: ctx[sq_sub, hd|den] += pexp_chunk.T @ v_aug
    (N=65 bf16 matmuls; fully-masked diagonal chunks skipped). Denominator
    from an appended ones-column; normalization = DVE reciprocal + one
    broadcast (stride-0) multiply into bf16.
  - PE transposes (via bf16 identity) flip normalized ctx back to [hd, sq]
    for the output projection out[sq, e] = ctx_T.T @ WoT, streamed to HBM.
  - Scheduling: flat (block, head) score-stage stream with ctx lagging one
    head so the in-order PE never waits on the exp latency; projection
    matmuls interleave as 2-matmul filler chunks, with deferrable work
    (oproj, late q chunks) parked in a second queue that drains into the
    exp-bound final block. The final block's oproj pre-accumulates Wo
    chunks c=0..2 into bf16 staging (re-added later via identity matmuls)
    and its last head masks on DVE and orders stages ascending, so the
    post-softmax tail is just c=3 + re-add + copy + DMA.
"""

import numpy as np
import concourse.bass as bass
import concourse.mybir as mybir
import concourse.tile as tile
from concourse.bass import broadcast_tensor_aps

F32 = mybir.dt.float32
BF16 = mybir.dt.bfloat16

B, S, D = 4, 2048, 1024
NH, HD = 8, 64          # per-core heads, head dim
EG = NH * HD            # 512: per-core projection width
P = 128
NSQ = S // 512          # 4 sq blocks of 512
NSK = S // 128          # 16 sk tiles of 128
NDO = D // P            # 8 d tiles


def _split_multi_waits(nc, cap_default=1):
    """This walrus build encodes at most 1 sem wait per instruction (2 for
    EventSemaphore); Tile's final drain carries one wait per active proc.
    Split excess waits onto preceding NOPs on the same engine."""
    n_split = 0
    for f in nc.m.functions:
        for blk in f.blocks:
            insts = blk.instructions
            new_list = []
            changed = False
            for i in insts:
                si = i.sync_info
                cap = 2 if i.opcode == "EventSemaphore" else cap_default
                if si is not None and len(si.on_wait) > cap:
                    waits = list(si.on_wait)
                    extra, keep = waits[:-cap], waits[-cap:]
                    for k, w in enumerate(extra):
                        nop = mybir.InstNoOp(
                            name=f"{i.name}_splitw{k}", ins=[], outs=[],
                            sync_info=mybir.SyncInfo(on_wait=[w], on_update=[]))
                        nop.engine = i.engine
                        new_list.append(nop)
                        n_split += 1
                    si.on_wait = keep
                    changed = True
                new_list.append(i)
            if changed:
                blk.instructions = new_list
    return n_split


def _build():
    nc = bass.Bass()
    xT = nc.dram_tensor("xT", [D, S], BF16, kind="ExternalInput")
    kvT = nc.dram_tensor("kvT", [D, S], BF16, kind="ExternalInput")
    wq_d = nc.dram_tensor("wq", [D, EG], BF16, kind="ExternalInput")
    wk_d = nc.dram_tensor("wk", [D, EG], BF16, kind="ExternalInput")
    wv_d = nc.dram_tensor("wv", [D, EG], BF16, kind="ExternalInput")
    wo_d = nc.dram_tensor("wo", [EG, D], BF16, kind="ExternalInput")
    id_d = nc.dram_tensor("ident", [P, P], BF16, kind="ExternalInput")
    out_d = nc.dram_tensor("out", [S, D], BF16, kind="ExternalOutput")

    xT_r = xT.rearrange("(do p) s -> p do s", p=P)
    kvT_r = kvT.rearrange("(do p) s -> p do s", p=P)

    with tile.TileContext(nc) as tc:
        with (
            tc.tile_pool(name="wpool", bufs=4) as wpool,
            tc.tile_pool(name="big", bufs=1) as big,
            tc.tile_pool(name="qpool", bufs=2) as qpool,
            tc.tile_pool(name="blk", bufs=3) as blkp,
            tc.tile_pool(name="pexp", bufs=18) as pexp_p,
            tc.tile_pool(name="csq", bufs=3) as csqp,
            tc.tile_pool(name="ctp", bufs=4) as ctp,
            tc.tile_pool(name="osb", bufs=8) as osbp,
            tc.tile_pool(name="oacc", bufs=8) as oaccp,
            tc.tile_pool(name="small", bufs=1) as small,
            tc.tile_pool(name="ps_acc", bufs=2, space="PSUM") as ps_acc,
            tc.tile_pool(name="ps_st", bufs=2, space="PSUM") as ps_st,
            tc.tile_pool(name="ps_ctx", bufs=2, space="PSUM") as ps_ctx,
        ):
            wk = wpool.tile([P, NDO, EG], BF16, tag="w", name="wk")
            wv = wpool.tile([P, NDO, EG], BF16, tag="w", name="wv")
            wq = wpool.tile([P, NDO, EG], BF16, tag="w", name="wq")
            kTs = big.tile([P, NSQ, S], BF16, tag="kts")     # [e%128, e//128, sk]
            v_aug = big.tile([P, NSK, NH, HD + 1], BF16, tag="vaug")
            ident = small.tile([P, P], BF16, tag="id")
            trimask = small.tile([P, P], BF16, tag="trimask")

            fill0 = nc.gpsimd.to_reg(0.0)
            # lower-triangular (keep sq >= sk) bf16 mask for the DVE-masked
            # final head
            nc.gpsimd.memset(trimask[:], 1.0)
            nc.gpsimd.affine_select(
                out=trimask[:], in_=trimask[:],
                compare_op=mybir.AluOpType.is_ge, fill=fill0, base=0,
                channel_multiplier=-1, pattern=[[1, P]])

            # Projection work is emitted as "filler" chunks (2 matmuls each)
            # interleaved into the attention stream to keep the in-order PE
            # busy through exp (ACT) latency windows.
            fillers = []       # must-drain before the consuming block (kv/qt)
            late = []          # deferrable (key, fn) — drained on PE slack

            def drain(n=1):
                for _ in range(n):
                    if fillers:
                        fillers.pop(0)()
                    elif late:
                        late.pop(0)[1]()
                    else:
                        return

            def force_late(key):
                """Emit (in order) all deferred chunks tagged `key` now."""
                rest, run = [], []
                for kv in late:
                    (run if kv[0] == key else rest).append(kv)
                late[:] = rest
                for _, fn in run:
                    fn()

            def chunked_group(n_mm, mm_fn, finish_fn, chunk=1, key=None,
                              q=None, use_st=False):
                state = {}
                if q is None:
                    q = fillers
                for c0 in range(0, n_mm, chunk):
                    def run(c0=c0):
                        if "ps" not in state:
                            if use_st:
                                state["ps"] = ps_st.tile(
                                    [P, 2, 512], F32, tag="st",
                                    name="psg")[:, 0, :]
                            else:
                                state["ps"] = ps_acc.tile(
                                    [P, 512], F32, tag="acc", name="psg")
                        for i in range(c0, min(c0 + chunk, n_mm)):
                            mm_fn(state["ps"], i)
                        if c0 + chunk >= n_mm:
                            finish_fn(state["ps"])
                    if q is late:
                        q.append((key, run))
                    else:
                        q.append(run)

            def queue_kv_block(skb, kvb=None, ks_first=False):
                if kvb is None:
                    kvb = blkp.tile([P, NDO, 512], BF16, tag="blk", name="kvb")
                    nc.sync.dma_start(
                        kvb[:], kvT_r[:, :, 512 * skb:512 * (skb + 1)])
                k_groups, v_groups = [], []
                for idx in range(4):
                    def mm_k(ps, do, idx=idx, kvb=kvb):
                        nc.tensor.matmul(
                            ps[:], wk[:, do, P * idx:P * (idx + 1)],
                            kvb[:, do, :],
                            start=(do == 0), stop=(do == NDO - 1))

                    def fin_k(ps, idx=idx, skb=skb):
                        nc.vector.tensor_copy(
                            kTs[:, idx, 512 * skb:512 * (skb + 1)], ps[:])

                    def mm_v(ps, do, idx=idx, kvb=kvb):
                        nc.tensor.matmul(
                            ps[:], kvb[:, do, P * idx:P * (idx + 1)],
                            wv[:, do, :],
                            start=(do == 0), stop=(do == NDO - 1))

                    def fin_v(ps, idx=idx, skb=skb):
                        nc.vector.tensor_copy(
                            v_aug[:, 4 * skb + idx, :, 0:HD],
                            ps[:].rearrange("p (h x) -> p h x", x=HD))

                    k_groups.append((mm_k, fin_k))
                    v_groups.append((mm_v, fin_v))
                if ks_first:
                    for mm, fin in k_groups + v_groups:
                        chunked_group(NDO, mm, fin)
                else:
                    # k gates the next block's first scores; v is only
                    # needed by ctx one head later — defer it so the block
                    # boundary drain doesn't delay the score/exp stream
                    for mmk, fink in k_groups:
                        chunked_group(NDO, mmk, fink)
                    for mmv, finv in v_groups:
                        chunked_group(NDO, mmv, finv, key=("v", skb), q=late)

            def queue_qt(sq_t, xb=None):
                if xb is None:
                    xb = blkp.tile([P, NDO, 512], BF16, tag="blk", name="xb")
                    nc.sync.dma_start(
                        xb[:], xT_r[:, :, 512 * sq_t:512 * (sq_t + 1)])
                qTs = qpool.tile([P, NSQ, 512], BF16, tag="qts", name="qTs")
                for eo in range(NSQ):
                    def mm_q(ps, do, eo=eo, xb=xb):
                        nc.tensor.matmul(
                            ps[:], wq[:, do, P * eo:P * (eo + 1)], xb[:, do, :],
                            start=(do == 0), stop=(do == NDO - 1))

                    def fin_q(ps, eo=eo, qTs=qTs):
                        nc.vector.tensor_copy(qTs[:, eo, :], ps[:])

                    # eo chunk is first consumed by head 2*eo: only eo=0 must
                    # precede the block; the rest defer as PE slack work,
                    # force-emitted before their consuming head
                    chunked_group(NDO, mm_q, fin_q, key=(sq_t, eo),
                                  q=None if eo == 0 else late)
                return qTs

            def queue_oproj_firsthalf(ctxT, accs):
                """Last block: accumulate Wo chunks c=0..2 (heads 0-5) into
                bf16 staging tiles during the ACT-bound window."""
                for sqs in range(4):
                    for es in range(2):
                        def mm_a(ps, c, es=es, sqs=sqs, ctxT=ctxT):
                            nc.tensor.matmul(
                                ps[:], ctxT[:, c, sqs, :],
                                wo[:, c, 512 * es:512 * (es + 1)],
                                start=(c == 0), stop=(c == 2))

                        def fin_a(ps, es=es, sqs=sqs, accs=accs):
                            acc = oaccp.tile([P, 512], BF16, tag="oacc",
                                             name="oacc")
                            with nc.allow_low_precision(reason="bf16 stage"):
                                nc.vector.tensor_copy(acc[:], ps[:])
                            accs[(es, sqs)] = acc

                        chunked_group(3, mm_a, fin_a, chunk=3, q=late)

            def queue_oproj_final(ctxT, sq0, accs):
                """Tail after the last exp: c=3 matmul + identity-matmul
                re-add of the staged half-sum, then copy (DVE/ACT
                alternating) + one merged DMA per row block."""
                for sqs in range(4):
                    shared = {}
                    for es in range(2):
                        def mm_b(ps, i, es=es, sqs=sqs, ctxT=ctxT,
                                 accs=accs):
                            if i == 0:
                                nc.tensor.matmul(
                                    ps[:], ctxT[:, 3, sqs, :],
                                    wo[:, 3, 512 * es:512 * (es + 1)],
                                    start=True, stop=False)
                            else:
                                # += staged first half via identity matmul
                                nc.tensor.matmul(
                                    ps[:], ident[:], accs[(es, sqs)][:],
                                    start=False, stop=True)

                        def fin_b(ps, es=es, sqs=sqs, sq0=sq0,
                                  shared=shared):
                            if "ot" not in shared:
                                shared["ot"] = osbp.tile([P, 2, 512], BF16,
                                                         tag="ot", name="ot")
                            ot = shared["ot"]
                            with nc.allow_low_precision(reason="bf16 out"):
                                if es == 0:
                                    nc.scalar.activation(
                                        ot[:, es, :], ps[:],
                                        mybir.ActivationFunctionType.Copy)
                                else:
                                    nc.vector.tensor_copy(ot[:, es, :], ps[:])
                            if es == 1:
                                nc.sync.dma_start(
                                    out_d[sq0 + P * sqs:sq0 + P * (sqs + 1), :],
                                    ot[:])

                        chunked_group(2, mm_b, fin_b, chunk=2, q=late,
                                      use_st=(es == 1))

            def queue_oproj(ctxT, sq0, last=False):
                for sqs in range(4):
                    shared = {}
                    for es in range(2):
                        def mm_o(ps, c, es=es, sqs=sqs, ctxT=ctxT):
                            nc.tensor.matmul(
                                ps[:], ctxT[:, c, sqs, :],
                                wo[:, c, 512 * es:512 * (es + 1)],
                                start=(c == 0), stop=(c == NSQ - 1))

                        def fin_o(ps, es=es, sqs=sqs, sq0=sq0, last=last,
                                  shared=shared):
                            # both es halves gather into one [P, 1024] tile:
                            # one output DMA per row block halves the
                            # per-DMA sequencer+DGE issue overhead
                            if "ot" not in shared:
                                shared["ot"] = osbp.tile([P, 2, 512], BF16,
                                                         tag="ot", name="ot")
                            ot = shared["ot"]
                            with nc.allow_low_precision(reason="bf16 out"):
                                if last and es == 0:
                                    # spread drain-critical tail copies over
                                    # the otherwise-idle ACT engine
                                    nc.scalar.activation(
                                        ot[:, es, :], ps[:],
                                        mybir.ActivationFunctionType.Copy)
                                else:
                                    nc.vector.tensor_copy(ot[:, es, :], ps[:])
                            if es == 1:
                                nc.sync.dma_start(
                                    out_d[sq0 + P * sqs:sq0 + P * (sqs + 1), :],
                                    ot[:])

                        # final block alternates psum pools (score psums are
                        # dead by then) for a 4-deep pipeline
                        chunked_group(NSQ, mm_o, fin_o, q=late,
                                      use_st=last and es == 1)

            # ---- startup: fine-grained DMAs so the first k-proj matmul can
            # start as soon as the first do-chunk of wk and kv block 0 land.
            # warm-up: zero a scratch tile and run throwaway matmuls so the
            # PE p-state ramp completes while the first input DMAs stream
            scr = small.tile([P, 512], BF16, tag="scr")
            nc.gpsimd.memset(scr[:], 0.0)
            for _ in range(10):
                wps = ps_acc.tile([P, 512], F32, tag="acc", name="wps")
                nc.tensor.matmul(wps[:], scr[:, 0:P], scr[:], start=True,
                                 stop=True)

            kvb0 = blkp.tile([P, NDO, 512], BF16, tag="blk", name="kvb0")
            for do in range(NDO):
                nc.sync.dma_start(
                    wk[:, do, :], wk_d.rearrange("(do p) e -> p do e", p=P)[:, do, :])
                nc.sync.dma_start(kvb0[:, do, :], kvT_r[:, do, 0:512])
            # DMA order matches PE consumption order: k (wk/kv), v (wv),
            # q (wq/xb) — each stream lands just before its matmuls need it
            for do in range(NDO):
                nc.sync.dma_start(
                    wv[:, do, :], wv_d.rearrange("(do p) e -> p do e", p=P)[:, do, :])
            nc.sync.dma_start(wq[:], wq_d.rearrange("(do p) e -> p do e", p=P))
            xb0 = blkp.tile([P, NDO, 512], BF16, tag="blk", name="xb0")
            nc.sync.dma_start(xb0[:], xT_r[:, :, 0:512])
            queue_kv_block(0, kvb=kvb0, ks_first=True)
            drain(len(fillers))
            qTs_cur = queue_qt(0, xb=xb0)
            drain(len(fillers))
            nc.sync.dma_start(ident[:], id_d[:])
            nc.gpsimd.memset(v_aug[:, :, :, HD], 1.0)

            wo = wpool.tile([P, NSQ, D], BF16, tag="w", name="wo")
            nc.sync.dma_start(wo[:], wo_d.rearrange("(do p) e -> p do e", p=P))

            # ---- flat (block, head) pipeline with ctx lagging one head
            qTs_by_block = {0: qTs_cur}

            def emit_stage(sq_t, h, skp, qTs):
                """Scores for sk pair `skp` of head h + exp (+ diag mask).
                Returns the pexp tile."""
                n_sk = 4 * (sq_t + 1)
                base = HD * (h % 2)
                eo = h // 2
                stp = ps_st.tile([P, 2, 512], F32, tag="st", name="stp")
                w0s = []
                for j in range(2):
                    sk_t = 2 * skp + j
                    i = sk_t - (n_sk - 4)  # diag-local index, >=0 on diagonal
                    w0 = P * i if i > 0 else 0
                    w0s.append(w0)
                    nc.tensor.matmul(
                        stp[:, j, w0:512],
                        kTs[base:base + HD, eo, P * sk_t:P * (sk_t + 1)],
                        qTs[base:base + HD, eo, w0:512],
                        start=True, stop=True)
                pexp_t = pexp_p.tile([P, 2, 512], BF16, tag="pexp",
                                     name="pexp_t")
                # one exp per sk pair, starting at the pair's smaller trim
                # offset: the narrower tile's leading columns hold junk that
                # gets exp'd but is never read (its ctx chunks are skipped)
                w0 = min(w0s)
                nc.scalar.activation(
                    pexp_t[:, :, w0:512], stp[:, :, w0:512],
                    mybir.ActivationFunctionType.Exp)
                for j in range(2):
                    sk_t = 2 * skp + j
                    i = sk_t - (n_sk - 4)
                    if i >= 0:
                        # exact-diagonal 128x128 subtile: keep sq >= sk
                        if (sq_t, h) == (NSQ - 1, NH - 1):
                            # last head: Pool's serialized affine chain would
                            # gate the tail — mask on DVE instead
                            with nc.allow_low_precision(reason="bf16 mask"):
                                nc.vector.tensor_tensor(
                                    pexp_t[:, j, P * i:P * (i + 1)],
                                    pexp_t[:, j, P * i:P * (i + 1)],
                                    trimask[:], mybir.AluOpType.mult)
                        else:
                            nc.gpsimd.affine_select(
                                out=pexp_t[:, j, P * i:P * (i + 1)],
                                in_=pexp_t[:, j, P * i:P * (i + 1)],
                                compare_op=mybir.AluOpType.is_ge,
                                fill=fill0, base=0,
                                channel_multiplier=-1, pattern=[[1, P]])
                return pexp_t

            def ctx_mm_list(sq_t, h, pexps):
                """s-major ctx matmuls for head h; returns (emitters, cps)."""
                n_sk = 4 * (sq_t + 1)
                cps = ps_ctx.tile([P, 4, HD + 1], F32, tag="ctxps", name="cps")
                mms = []
                for s in range(4):
                    sk_hi = min(n_sk - 1, n_sk - 4 + s)
                    # ascending sk: the affine-masked diagonal chunk comes
                    # last, so its GPSIMD mask has drained by then
                    for sk_t in range(0, sk_hi + 1):
                        def mm(s=s, sk_t=sk_t, sk_hi=sk_hi):
                            skp, j = divmod(sk_t, 2)
                            nc.tensor.matmul(
                                cps[:, s, :],
                                pexps[skp][:, j, P * s:P * (s + 1)],
                                v_aug[:, sk_t, h, :],
                                start=(sk_t == 0), stop=(sk_t == sk_hi))
                        mms.append(mm)
                return mms, cps

            def emit_norm(h, cps):
                """DVE reciprocal + broadcast multiply -> bf16 ctx_sq tile."""
                rcps = csqp.tile([P, 4, 1], F32, tag="rcp", name="rcps")
                ctxsq = csqp.tile([P, 4, HD], BF16, tag="csq", name="ctxsq")
                with nc.allow_low_precision(reason="softmax recip + bf16 ctx"):
                    nc.vector.reciprocal(rcps[:, :, 0], cps[:, :, HD])
                    b0, b1 = broadcast_tensor_aps(cps[:, :, 0:HD], rcps[:, :, 0:1])
                    nc.vector.tensor_tensor(ctxsq[:], b0, b1,
                                            mybir.AluOpType.mult)
                return ctxsq

            def emit_tps(h, ctxsq, ctxT):
                """PE transposes of normalized ctx + DVE copy into ctx_T."""
                tps = ps_ctx.tile([HD, 4, P], BF16, tag="ctxps", name="tps")
                for s in range(4):
                    nc.tensor.transpose(tps[:, s, :], ctxsq[:, s, :], ident[:])
                base = HD * (h % 2)
                nc.vector.tensor_copy(ctxT[base:base + HD, h // 2, :, :], tps[:])

            # stream state
            oproj_accs = {}      # (es, sqs) -> staged c=0..2 half sums
            prev = None          # (sq_t, h, pexps, qTs) waiting for ctx
            pending = []         # deferred emitters (transposes etc.)
            ctxT_cur = None
            ctxT_by_block = {}

            def flush_prev(stages_budget=None, inter_stage=None):
                pass

            seq = [(b, h) for b in range(NSQ) for h in range(NH)]
            for b, h in seq:
                sq_t = b
                n_skp = 2 * (sq_t + 1)
                if h == 0:
                    # block boundary: kv/qt fillers for this block must be
                    # fully emitted before its first score matmul
                    drain(len(fillers))
                    if b not in qTs_by_block:
                        qTs_by_block[b] = queue_qt(b)
                        drain(len(fillers))
                    ctxT_cur = ctp.tile([P, NSQ, 4, P], BF16, tag="ctxT",
                                        name="ctxT")
                    ctxT_by_block[b] = ctxT_cur
                    if b + 1 < NSQ:
                        queue_kv_block(b + 1)
                        qTs_by_block[b + 1] = queue_qt(b + 1)
                qTs = qTs_by_block[b]
                force_late((b, h // 2))
                if h == 1:
                    # ctx(b, 0) interleaves into this head's stages: its v
                    # chunks must be fully emitted first
                    force_late(("v", b))

                # ctx work for the previous (block, head), interleaved into
                # this head's score stages
                if prev is not None:
                    p_sqt, p_h, p_pexps, _ = prev
                    cmms, cps = ctx_mm_list(p_sqt, p_h, p_pexps)
                else:
                    cmms, cps = [], None
                per_stage = (len(cmms) + n_skp - 1) // n_skp if cmms else 0

                # last head of the stream: ascending stages, so its ctx
                # (ascending sk) consumes exps in production order and the
                # tail doesn't HOL-wait on the final exp
                if (b, h) == (NSQ - 1, NH - 1):
                    skp_order = list(range(n_skp))
                else:
                    skp_order = list(range(n_skp - 1, -1, -1))
                if b == NSQ - 1 and h == 7:
                    # most of the final block's oproj contraction (c=0..2)
                    # can run in the ACT-bound window once heads 0-5 are
                    # transposed (tps(5) is emitted by this head's first
                    # pending pop, which precedes the first drain); only
                    # c=3 + a staged re-add remain after the last exp
                    queue_oproj_firsthalf(ctxT_cur, oproj_accs)
                pexps = {}
                for si, skp in enumerate(skp_order):
                    pexps[skp] = emit_stage(sq_t, h, skp, qTs)
                    half = (per_stage + 1) // 2
                    for _ in range(half):
                        if cmms:
                            cmms.pop(0)()
                    drain(1)
                    for _ in range(per_stage - half):
                        if cmms:
                            cmms.pop(0)()
                    if pending:
                        pending.pop(0)()
                    drain(7 if b == 0 else (1 if b == 1 else 0))
                while cmms:
                    cmms.pop(0)()
                if prev is not None:
                    p_sqt, p_h, p_pexps, p_ctxT = prev
                    ctxsq = emit_norm(p_h, cps)
                    pending.append(
                        lambda p_h=p_h, ctxsq=ctxsq, p_ctxT=p_ctxT: (
                            emit_tps(p_h, ctxsq, p_ctxT)))
                    if p_h == NH - 1:
                        # previous block complete: queue its output projection
                        while pending:
                            pending.pop(0)()
                        queue_oproj(p_ctxT, 512 * p_sqt)
                prev = (sq_t, h, pexps, ctxT_cur)

            # ---- tail: ctx for the last head, then final oproj
            p_sqt, p_h, p_pexps, p_ctxT = prev
            cmms, cps = ctx_mm_list(p_sqt, p_h, p_pexps)
            ci = 0
            while cmms:
                cmms.pop(0)()
                ci += 1
                if ci % 2 == 0:
                    drain(1)
            ctxsq = emit_norm(p_h, cps)
            while pending:
                pending.pop(0)()
            drain(len(fillers) + len(late))
            emit_tps(p_h, ctxsq, p_ctxT)
            queue_oproj_final(p_ctxT, 512 * p_sqt, oproj_accs)
            drain(len(fillers) + len(late))

    _split_multi_waits(nc)
    return nc


_CACHED = None


def _get_nc():
    global _CACHED
    if _CACHED is None:
        _CACHED = _build()
    return _CACHED


def _make_in_maps(x, kv, Wq, Wk, Wv, Wo):
    import ml_dtypes
    bf = ml_dtypes.bfloat16
    x = np.asarray(x, np.float32)
    kv = np.asarray(kv, np.float32)
    WqT = (np.asarray(Wq, np.float32).T / np.sqrt(np.float32(HD))).astype(bf)
    WkT = np.asarray(Wk, np.float32).T.astype(bf)
    WvT = np.asarray(Wv, np.float32).T.astype(bf)
    WoT = np.asarray(Wo, np.float32).T.astype(bf)
    ident = np.eye(P, dtype=bf)
    in_maps = []
    for c in range(8):
        b, g = c // 2, c % 2
        e0 = g * EG
        in_maps.append({
            "xT": np.ascontiguousarray(x[b].T).astype(bf),
            "kvT": np.ascontiguousarray(kv[b].T).astype(bf),
            "wq": np.ascontiguousarray(WqT[:, e0:e0 + EG]),
            "wk": np.ascontiguousarray(WkT[:, e0:e0 + EG]),
            "wv": np.ascontiguousarray(WvT[:, e0:e0 + EG]),
            "wo": np.ascontiguousarray(WoT[e0:e0 + EG, :]),
            "ident": ident,
        })
    return in_maps


_RUNNER = None


def _get_runner():
    """Persistent jitted 8-core runner (mirrors bass2jax.run_bass_via_pjrt's
    multi-core path, but reusable across kernel() calls so the NEFF is
    compiled once)."""
    global _RUNNER
    if _RUNNER is not None:
        return _RUNNER
    import jax
    from jax.sharding import Mesh, PartitionSpec, NamedSharding
    from jax.experimental.shard_map import shard_map
    from concourse import bass2jax

    nc = _get_nc()
    bass2jax.install_neuronx_cc_hook()
    pname = nc.partition_id_tensor.name if nc.partition_id_tensor else None
    in_names, out_names, out_avals, zero_outs = [], [], [], []
    for alloc in nc.m.functions[0].allocations:
        if not isinstance(alloc, mybir.MemoryLocationSet):
            continue
        name = alloc.memorylocations[0].name
        if alloc.kind == "ExternalInput":
            if name != pname:
                in_names.append(name)
        elif alloc.kind == "ExternalOutput":
            out_names.append(name)
            shape = tuple(alloc.tensor_shape)
            dtype = mybir.dt.np(alloc.dtype)
            out_avals.append(jax.core.ShapedArray(shape, dtype))
            zero_outs.append(np.zeros(shape, dtype))
    n_params = len(in_names)
    all_in = in_names + out_names + ([pname] if pname else [])

    def _body(*args):
        operands = list(args)
        if pname:
            operands.append(bass2jax.partition_id_tensor())
        outs = bass2jax._bass_exec_p.bind(
            *operands, out_avals=tuple(out_avals), in_names=tuple(all_in),
            out_names=tuple(out_names), lowering_input_output_aliases=(),
            sim_require_finite=True, sim_require_nnan=True, nc=nc)
        return tuple(outs)

    n_cores = 8
    try:
        devices = jax.devices("axon")[:n_cores]
    except Exception:
        devices = jax.devices()[:n_cores]
    assert len(devices) == n_cores, f"need {n_cores} neuron cores, have {devices}"
    mesh = Mesh(np.asarray(devices), ("core",))
    donate = tuple(range(n_params, n_params + len(out_names)))
    fn = jax.jit(shard_map(
        _body, mesh=mesh,
        in_specs=(PartitionSpec("core"),) * (n_params + len(out_names)),
        out_specs=(PartitionSpec("core"),) * len(out_names), check_rep=False),
        donate_argnums=donate, keep_unused=True)
    sh = NamedSharding(mesh, PartitionSpec("core"))
    _RUNNER = (fn, in_names, out_names, zero_outs, sh, n_cores)
    return _RUNNER


def _run(x, kv, Wq, Wk, Wv, Wo):
    import jax

    in_maps = _make_in_maps(x, kv, Wq, Wk, Wv, Wo)
    fn, in_names, out_names, zero_outs, sh, n_cores = _get_runner()
    concat_in = [
        np.concatenate([m[name] for m in in_maps], axis=0) for name in in_names
    ]
    concat_z = [np.concatenate([z] * n_cores, axis=0) for z in zero_outs]
    args = [jax.device_put(a, sh) for a in concat_in + concat_z]
    outs = fn(*args)
    per_core = np.asarray(outs[out_names.index("out")]).reshape(
        n_cores, S, D).astype(np.float32)
    out = np.empty((B, S, D), np.float32)
    for b in range(B):
        out[b] = per_core[2 * b] + per_core[2 * b + 1]
    return out


def kernel(x, kv, mask, Wq, Wk, Wv, Wo):
    return _run(x, kv, Wq, Wk, Wv, Wo)



# revision 29
# speedup vs baseline: 1.0727x; 1.0727x over previous
"""Multi-head attention (B=4, S=2048, D=1024, H=16, causal) on 8 TRN2 cores.

Sharding: data-parallel over batch (4) x tensor-parallel over heads (2 groups
of 8). Core c handles batch c//2, head group c%2. Each core computes a partial
output projection (its 8 heads through its Wo rows); the host sums the two
partials per batch (with a 2^-8 unscale compensating the fp8 weight scale).

Device algorithm (per core, fp32 psum accumulation):
  - Projections and the output projection run as TWO-TERM fp8 matmuls in
    DoubleRow perf mode (0.5 cy/row): each bf16 operand a is decomposed as
    a ~= a_hi + a_lo (both e4m3); the product keeps the hi.hi, lo.hi, hi.lo
    terms (lo.lo dropped, ~1e-3 relative), packed as 3 DoubleRow
    instructions per 2 contraction chunks (0.75x the bf16 PE cost, better
    accuracy than plain bf16). Weights are pre-scaled x256 into fp8 normal
    range on the host; compensation: exp(score * 2^-16), a 256-valued
    denominator column, and a 2^-8 host unscale of the output.
  - Host supplies hi/lo fp8 activations xT8/kvT8 [D, {h,h,l}, S] (hi
    duplicated so DoubleRow moving APs never need stride-0 dims) and fp8
    weight shards wq8/wk8 [D, {h,l}, EG], wv8 [D, {h,h,l}, EG],
    wo8 [EG, {h,h,l}, D] (q-scale 1/sqrt(hd) folded into Wq).
  - kT [e, sk] and v [sk, e] from projection matmuls; qT per sq block.
  - Scores (bf16) TRANSPOSED: ST[sk, sq] = kT_h.T @ qT_h per 128-wide sk
    tile, width trimmed at the causal diagonal. exp on ACT (scale 2^-16)
    into bf16 pexp tiles; exact-diagonal subtile masked by GPSIMD
    affine_select (DVE trimask multiply on the final head).
  - ctx in [sq, hd] orientation: ctx[sq_sub, hd|den] += pexp_chunk.T @ v_aug
    (bf16, fully-masked diagonal chunks skipped). Denominator from an
    appended 256-column; normalization = DVE reciprocal + one broadcast
    multiply into per-head-PAIR bf16 tiles [sq, 4, 2*hd].
  - Head-pair ctx tiles are transposed to [hd2, sq] by the DMA XBAR
    (dma_start_transpose, off the PE), then split hi/lo to fp8 (Pool copy +
    DVE subtract) for the fp8 output projection. The final block's last
    pair is PE-transposed instead to keep the drain tail short, and its
    c2/c3 Wo chunks stay bf16 (wo_bf) so no split sits on the tail.
  - Scheduling: flat (block, head) score-stage stream with ctx lagging one
    head; projection work interleaves as 2-matmul filler chunks, with
    deferrable work (v-proj, late q chunks, splits, oproj) in a second
    queue drained on PE slack. The final block pre-accumulates oproj
    chunks c0/c1 (fp8) + c2 (bf16) into bf16 staging re-added via identity
    matmuls, leaving just c3 + re-add + copy + DMA after the last exp.
"""

import numpy as np
import concourse.bass as bass
import concourse.mybir as mybir
import concourse.tile as tile
from concourse.bass import broadcast_tensor_aps

F32 = mybir.dt.float32
BF16 = mybir.dt.bfloat16
FP8 = mybir.dt.float8e4
DR = mybir.MatmulPerfMode.DoubleRow

B, S, D = 4, 2048, 1024
import os
DRAIN2 = [int(x) for x in os.environ.get('DRAIN2', '7,1,2,1').split(',')]
WARMUP = int(os.environ.get('WARMUP', '10'))
NH, HD = 8, 64          # per-core heads, head dim
EG = NH * HD            # 512: per-core projection width
P = 128
NSQ = S // 512          # 4 sq blocks of 512
NSK = S // 128          # 16 sk tiles of 128
NDO = D // P            # 8 d tiles
WSCALE = 256.0          # fp8 weight pre-scale (host); pow2, exactly undone
EXP_SCALE = 2.0 ** -16  # undoes q,k weight scales inside the exp
DEN_VAL = 256.0         # denominator column value: undoes the v weight scale


def _split_multi_waits(nc, cap_default=1):
    """This walrus build encodes at most 1 sem wait per instruction (2 for
    EventSemaphore); Tile's final drain carries one wait per active proc.
    Split excess waits onto preceding NOPs on the same engine."""
    n_split = 0
    for f in nc.m.functions:
        for blk in f.blocks:
            insts = blk.instructions
            new_list = []
            changed = False
            for i in insts:
                si = i.sync_info
                cap = 2 if i.opcode == "EventSemaphore" else cap_default
                if si is not None and len(si.on_wait) > cap:
                    waits = list(si.on_wait)
                    extra, keep = waits[:-cap], waits[-cap:]
                    for k, w in enumerate(extra):
                        nop = mybir.InstNoOp(
                            name=f"{i.name}_splitw{k}", ins=[], outs=[],
                            sync_info=mybir.SyncInfo(on_wait=[w], on_update=[]))
                        nop.engine = i.engine
                        new_list.append(nop)
                        n_split += 1
                    si.on_wait = keep
                    changed = True
                new_list.append(i)
            if changed:
                blk.instructions = new_list
    return n_split


def _build():
    nc = bass.Bass()
    xT8 = nc.dram_tensor("xT8", [2 * D, S], FP8, kind="ExternalInput")
    kvT8 = nc.dram_tensor("kvT8", [2 * D, S], FP8, kind="ExternalInput")
    wq_d = nc.dram_tensor("wq8", [D, 2, EG], FP8, kind="ExternalInput")
    wk_d = nc.dram_tensor("wk8", [D, 2, EG], FP8, kind="ExternalInput")
    wv_d = nc.dram_tensor("wv8", [D, 2, EG], FP8, kind="ExternalInput")
    wo_d = nc.dram_tensor("wo8", [EG, 2, D], FP8, kind="ExternalInput")
    wobf_d = nc.dram_tensor("wobf", [2 * P, D], BF16, kind="ExternalInput")
    id_d = nc.dram_tensor("ident", [P, P], BF16, kind="ExternalInput")
    out_d = nc.dram_tensor("out", [S, D], BF16, kind="ExternalOutput")

    xT_r = xT8.rearrange("(t do p) s -> p t do s", p=P, do=NDO)
    kvT_r = kvT8.rearrange("(t do p) s -> p t do s", p=P, do=NDO)

    with tile.TileContext(nc) as tc:
        with (
            tc.tile_pool(name="wpool", bufs=4) as wpool,
            tc.tile_pool(name="big", bufs=1) as big,
            tc.tile_pool(name="qpool", bufs=2) as qpool,
            tc.tile_pool(name="blk", bufs=3) as blkp,
            tc.tile_pool(name="pexp", bufs=17) as pexp_p,
            tc.tile_pool(name="csq", bufs=3) as csqp,
            tc.tile_pool(name="pairp", bufs=3) as pairp,
            tc.tile_pool(name="ctp", bufs=3) as ctp,
            tc.tile_pool(name="ct8", bufs=2) as ct8p,
            tc.tile_pool(name="osb", bufs=8) as osbp,
            tc.tile_pool(name="oacc", bufs=8) as oaccp,
            tc.tile_pool(name="small", bufs=1) as small,
            tc.tile_pool(name="ps_acc", bufs=2, space="PSUM") as ps_acc,
            tc.tile_pool(name="ps_st", bufs=2, space="PSUM") as ps_st,
            tc.tile_pool(name="ps_ctx", bufs=2, space="PSUM") as ps_ctx,
        ):
            wk = wpool.tile([P, NDO, 2, EG], FP8, tag="w", name="wk")
            wv = wpool.tile([P, NDO, 2, EG], FP8, tag="w", name="wv")
            wq = wpool.tile([P, NDO, 2, EG], FP8, tag="w", name="wq")
            kTs = big.tile([P, NSQ, S], BF16, tag="kts")     # [e%128, e//128, sk]
            v_aug = big.tile([P, NSK, NH, HD + 1], BF16, tag="vaug")
            ident = small.tile([P, P], BF16, tag="id")
            trimask = small.tile([P, P], BF16, tag="trimask")

            fill0 = nc.gpsimd.to_reg(0.0)
            # lower-triangular (keep sq >= sk) bf16 mask for the DVE-masked
            # final head
            nc.gpsimd.memset(trimask[:], 1.0)
            nc.gpsimd.affine_select(
                out=trimask[:], in_=trimask[:],
                compare_op=mybir.AluOpType.is_ge, fill=fill0, base=0,
                channel_multiplier=-1, pattern=[[1, P]])

            # Projection work is emitted as "filler" chunks (2 matmuls each)
            # interleaved into the attention stream to keep the in-order PE
            # busy through exp (ACT) latency windows.
            fillers = []       # must-drain before the consuming block (kv/qt)
            late = []          # deferrable (key, fn) — drained on PE slack

            def drain(n=1):
                for _ in range(n):
                    if fillers:
                        fillers.pop(0)()
                    elif late:
                        late.pop(0)[1]()
                    else:
                        return

            def force_late(key):
                """Emit (in order) all deferred chunks tagged `key` now."""
                rest, run = [], []
                for kv in late:
                    (run if kv[0] == key else rest).append(kv)
                late[:] = rest
                for _, fn in run:
                    fn()

            def chunked_group(n_mm, mm_fn, finish_fn, chunk=1, key=None,
                              q=None, use_st=False):
                state = {}
                if q is None:
                    q = fillers
                for c0 in range(0, n_mm, chunk):
                    def run(c0=c0):
                        if "ps" not in state:
                            if use_st:
                                state["ps"] = ps_st.tile(
                                    [P, 2, 512], F32, tag="st",
                                    name="psg")[:, 0, :]
                            else:
                                state["ps"] = ps_acc.tile(
                                    [P, 512], F32, tag="acc", name="psg")
                        for i in range(c0, min(c0 + chunk, n_mm)):
                            mm_fn(state["ps"], i)
                        if c0 + chunk >= n_mm:
                            finish_fn(state["ps"])
                    if q is late:
                        q.append((key, run))
                    else:
                        q.append(run)

            def bcast2(ap):
                # duplicate a [K, N] AP into [K, 2, N] with a stride-0 dim:
                # DoubleRow reads the same moving tile for both k-slots
                return bass.AP(ap.tensor, ap.offset,
                               [list(ap.ap[0]), [0, 2],
                                *[list(d) for d in ap.ap[1:]]])

            # -- two-term fp8 DoubleRow emission: 12 insts contract D=1024.
            # Per do-pair dp: A(2dp), A(2dp+1), B(dp) where
            #   A(do): stationary (s_hi[do], s_lo[do]) x moving (m_hi, m_hi)
            #   B(dp): stationary (s_hi[2dp], s_hi[2dp+1]) x moving m_lo pair
            # Stationary layout {h,l} (2 slots), moving layout {h,h,l}.
            def dr_mm(ps, i, n_mm, stat_a, stat_b, mov_a, mov_b):
                dp, r = divmod(i, 3)
                if r < 2:
                    do = 2 * dp + r
                    nc.tensor.matmul(ps[:], stat_a(do), mov_a(do),
                                     start=(i == 0), stop=(i == n_mm - 1),
                                     perf_mode=DR)
                else:
                    nc.tensor.matmul(ps[:], stat_b(dp), mov_b(dp),
                                     start=(i == 0), stop=(i == n_mm - 1),
                                     perf_mode=DR)

            def queue_kv_block(skb, kvb=None, ks_first=False):
                if kvb is None:
                    kvb = blkp.tile([P, 3, NDO, 512], FP8, tag="blk",
                                    name="kvb")
                    nc.sync.dma_start(
                        kvb[:], kvT_r[:, :, :, 512 * skb:512 * (skb + 1)])
                k_groups, v_groups = [], []
                for idx in range(4):
                    cs = slice(P * idx, P * (idx + 1))

                    def mm_k(ps, i, cs=cs, kvb=kvb):
                        dr_mm(ps, i, 12,
                              lambda do: wk[:, do, :, cs],
                              lambda dp: wk[:, 2 * dp:2 * dp + 2, 0, cs],
                              lambda do: bcast2(kvb[:, 0, do, :]),
                              lambda dp: kvb[:, 1, 2 * dp:2 * dp + 2, :])

                    def fin_k(ps, idx=idx, skb=skb):
                        nc.vector.tensor_copy(
                            kTs[:, idx, 512 * skb:512 * (skb + 1)], ps[:])

                    def mm_v(ps, i, cs=cs, kvb=kvb):
                        dr_mm(ps, i, 12,
                              lambda do: kvb[:, 0:2, do, cs],
                              lambda dp: kvb[:, 0, 2 * dp:2 * dp + 2, cs],
                              lambda do: bcast2(wv[:, do, 0, :]),
                              lambda dp: wv[:, 2 * dp:2 * dp + 2, 1, :])

                    def fin_v(ps, idx=idx, skb=skb):
                        nc.vector.tensor_copy(
                            v_aug[:, 4 * skb + idx, :, 0:HD],
                            ps[:].rearrange("p (h x) -> p h x", x=HD))

                    k_groups.append((mm_k, fin_k))
                    v_groups.append((mm_v, fin_v))
                if ks_first:
                    for mm, fin in k_groups + v_groups:
                        chunked_group(12, mm, fin, chunk=2)
                else:
                    # k gates the next block's first scores; v is only
                    # needed by ctx one head later — defer it so the block
                    # boundary drain doesn't delay the score/exp stream
                    for mmk, fink in k_groups:
                        chunked_group(12, mmk, fink, chunk=2)
                    for mmv, finv in v_groups:
                        chunked_group(12, mmv, finv, chunk=2,
                                      key=("v", skb), q=late)

            def queue_qt(sq_t, xb=None):
                if xb is None:
                    xb = blkp.tile([P, 2, NDO, 512], FP8, tag="blk", name="xb")
                    nc.sync.dma_start(
                        xb[:], xT_r[:, :, :, 512 * sq_t:512 * (sq_t + 1)])
                qTs = qpool.tile([P, NSQ, 512], BF16, tag="qts", name="qTs")
                for eo in range(NSQ):
                    cs = slice(P * eo, P * (eo + 1))

                    def mm_q(ps, i, cs=cs, xb=xb):
                        dr_mm(ps, i, 12,
                              lambda do: wq[:, do, :, cs],
                              lambda dp: wq[:, 2 * dp:2 * dp + 2, 0, cs],
                              lambda do: bcast2(xb[:, 0, do, :]),
                              lambda dp: xb[:, 1, 2 * dp:2 * dp + 2, :])

                    def fin_q(ps, eo=eo, qTs=qTs):
                        nc.vector.tensor_copy(qTs[:, eo, :], ps[:])

                    # eo chunk is first consumed by head 2*eo: only eo=0 must
                    # precede the block; the rest defer as PE slack work,
                    # force-emitted before their consuming head
                    chunked_group(12, mm_q, fin_q, chunk=2, key=(sq_t, eo),
                                  q=None if eo == 0 else late)
                return qTs

            def run_ctx_split(ctxT, ctxT8, c):
                """hi/lo fp8 split of transposed-ctx chunk c (Pool + DVE)."""
                with nc.allow_low_precision(reason="fp8 split"):
                    nc.gpsimd.tensor_copy(
                        ctxT8[:, c, 0, :, :], ctxT[:, c, :, :])
                    nc.vector.tensor_tensor(
                        ctxT8[:, c, 1, :, :], ctxT[:, c, :, :],
                        ctxT8[:, c, 0, :, :], mybir.AluOpType.subtract)

            def queue_ctx_split(ctxT, ctxT8, c):
                late.append((("split", c),
                             lambda: run_ctx_split(ctxT, ctxT8, c)))

            # fp8 oproj contraction: 6 DoubleRow insts cover chunks c0..c3
            # (order A0 A1 B01 A2 A3 B23); bf16 variants take wobf chunks.
            def mm_oproj(ps, i, es, sqs, ctxT8, start, stop):
                seq = [(0, 0), (0, 1), (1, 0), (0, 2), (0, 3), (1, 1)]
                kind, j = seq[i]
                if kind == 0:
                    nc.tensor.matmul(
                        ps[:], ctxT8[:, j, :, sqs, :],
                        bcast2(wo[:, j, 0, 512 * es:512 * (es + 1)]),
                        start=start, stop=stop, perf_mode=DR)
                else:
                    nc.tensor.matmul(
                        ps[:], ctxT8[:, 2 * j:2 * j + 2, 0, sqs, :],
                        wo[:, 2 * j:2 * j + 2, 1, 512 * es:512 * (es + 1)],
                        start=start, stop=stop, perf_mode=DR)

            def queue_oproj(ctxT8, sq0):
                for sqs in range(4):
                    shared = {}
                    for es in range(2):
                        def mm_o(ps, i, es=es, sqs=sqs, ctxT8=ctxT8):
                            mm_oproj(ps, i, es, sqs, ctxT8,
                                     start=(i == 0), stop=(i == 5))

                        def fin_o(ps, es=es, sqs=sqs, sq0=sq0,
                                  shared=shared):
                            # both es halves gather into one [P, 1024] tile:
                            # one output DMA per row block halves the
                            # per-DMA sequencer+DGE issue overhead
                            if "ot" not in shared:
                                shared["ot"] = osbp.tile([P, 2, 512], BF16,
                                                         tag="ot", name="ot")
                            ot = shared["ot"]
                            with nc.allow_low_precision(reason="bf16 out"):
                                nc.vector.tensor_copy(ot[:, es, :], ps[:])
                            if es == 1:
                                nc.sync.dma_start(
                                    out_d[sq0 + P * sqs:sq0 + P * (sqs + 1), :],
                                    ot[:])

                        chunked_group(6, mm_o, fin_o, q=late)

            def queue_oproj_firsthalf(ctxT, ctxT8, accs):
                """Last block: accumulate chunks c0,c1 (fp8) + c2 (bf16) into
                bf16 staging tiles during the ACT-bound window."""
                for sqs in range(4):
                    for es in range(2):
                        def mm_a(ps, i, es=es, sqs=sqs, ctxT=ctxT,
                                 ctxT8=ctxT8):
                            if i < 3:
                                mm_oproj(ps, i, es, sqs, ctxT8,
                                         start=(i == 0), stop=False)
                            else:
                                nc.tensor.matmul(
                                    ps[:], ctxT[:, 2, sqs, :],
                                    wobf[:, 0, 512 * es:512 * (es + 1)],
                                    start=False, stop=True)

                        def fin_a(ps, es=es, sqs=sqs, accs=accs):
                            acc = oaccp.tile([P, 512], BF16, tag="oacc",
                                             name="oacc")
                            with nc.allow_low_precision(reason="bf16 stage"):
                                nc.vector.tensor_copy(acc[:], ps[:])
                            accs[(es, sqs)] = acc

                        chunked_group(4, mm_a, fin_a, chunk=4, q=late)

            def queue_oproj_final(ctxT, sq0, accs):
                """Tail after the last exp: c3 bf16 matmul + identity-matmul
                re-add of the staged sum, then copy (DVE/ACT alternating) +
                one merged DMA per row block."""
                for sqs in range(4):
                    shared = {}
                    for es in range(2):
                        def mm_b(ps, i, es=es, sqs=sqs, ctxT=ctxT,
                                 accs=accs):
                            if i == 0:
                                nc.tensor.matmul(
                                    ps[:], ctxT[:, 3, sqs, :],
                                    wobf[:, 1, 512 * es:512 * (es + 1)],
                                    start=True, stop=False)
                            else:
                                # += staged first half via identity matmul
                                nc.tensor.matmul(
                                    ps[:], ident[:], accs[(es, sqs)][:],
                                    start=False, stop=True)

                        def fin_b(ps, es=es, sqs=sqs, sq0=sq0,
                                  shared=shared):
                            if "ot" not in shared:
                                shared["ot"] = osbp.tile([P, 2, 512], BF16,
                                                         tag="ot", name="ot")
                            ot = shared["ot"]
                            with nc.allow_low_precision(reason="bf16 out"):
                                if es == 0:
                                    nc.scalar.activation(
                                        ot[:, es, :], ps[:],
                                        mybir.ActivationFunctionType.Copy)
                                else:
                                    nc.vector.tensor_copy(ot[:, es, :], ps[:])
                            if es == 1:
                                nc.sync.dma_start(
                                    out_d[sq0 + P * sqs:sq0 + P * (sqs + 1), :],
                                    ot[:])

                        chunked_group(2, mm_b, fin_b, chunk=2, q=late,
                                      use_st=(es == 1))

            # ---- startup: fine-grained DMAs so the first k-proj matmul can
            # start as soon as the first do-chunk of wk and kv block 0 land.
            # warm-up: zero a scratch tile and run throwaway matmuls so the
            # PE p-state ramp completes while the first input DMAs stream
            scr = small.tile([P, 512], BF16, tag="scr")
            nc.gpsimd.memset(scr[:], 0.0)
            for _ in range(WARMUP):
                wps = ps_acc.tile([P, 512], F32, tag="acc", name="wps")
                nc.tensor.matmul(wps[:], scr[:, 0:P], scr[:], start=True,
                                 stop=True)

            kvb0 = blkp.tile([P, 2, NDO, 512], FP8, tag="blk", name="kvb0")
            wk_r = wk_d.rearrange("(do p) t e -> p do t e", p=P)
            wv_r = wv_d.rearrange("(do p) t e -> p do t e", p=P)
            wq_r = wq_d.rearrange("(do p) t e -> p do t e", p=P)
            # single DMA device: transfers land strictly in issue order, so
            # issue in PE consumption order: k (wk/kv), v (wv), q (wq/xb)
            for do in range(NDO):
                nc.sync.dma_start(wk[:, do, :, :], wk_r[:, do, :, :])
                nc.sync.dma_start(kvb0[:, :, do, :], kvT_r[:, :, do, 0:512])
            for do in range(NDO):
                nc.sync.dma_start(wv[:, do, :, :], wv_r[:, do, :, :])
            nc.scalar.dma_start(wq[:], wq_r[:])
            xb0 = blkp.tile([P, 2, NDO, 512], FP8, tag="blk", name="xb0")
            nc.scalar.dma_start(xb0[:], xT_r[:, :, :, 0:512])
            queue_kv_block(0, kvb=kvb0, ks_first=True)
            drain(len(fillers))
            qTs_cur = queue_qt(0, xb=xb0)
            drain(len(fillers))
            nc.scalar.dma_start(ident[:], id_d[:])
            nc.gpsimd.memset(v_aug[:, :, :, HD], DEN_VAL)

            wo = wpool.tile([P, NSQ, 2, D], FP8, tag="w", name="wo")
            nc.scalar.dma_start(
                wo[:], wo_d.rearrange("(c p) t e -> p c t e", p=P))
            wobf = wpool.tile([P, 2, D], BF16, tag="w", name="wobf")
            nc.scalar.dma_start(
                wobf[:], wobf_d.rearrange("(c p) e -> p c e", p=P))

            # ---- flat (block, head) pipeline with ctx lagging one head
            qTs_by_block = {0: qTs_cur}

            def emit_stage(sq_t, h, skp, qTs):
                """Scores for sk pair `skp` of head h + exp (+ diag mask).
                Returns the pexp tile."""
                n_sk = 4 * (sq_t + 1)
                base = HD * (h % 2)
                eo = h // 2
                stp = ps_st.tile([P, 2, 512], F32, tag="st", name="stp")
                w0s = []
                for j in range(2):
                    sk_t = 2 * skp + j
                    i = sk_t - (n_sk - 4)  # diag-local index, >=0 on diagonal
                    w0 = P * i if i > 0 else 0
                    w0s.append(w0)
                    nc.tensor.matmul(
                        stp[:, j, w0:512],
                        kTs[base:base + HD, eo, P * sk_t:P * (sk_t + 1)],
                        qTs[base:base + HD, eo, w0:512],
                        start=True, stop=True)
                pexp_t = pexp_p.tile([P, 2, 512], BF16, tag="pexp",
                                     name="pexp_t")
                # one exp per sk pair, starting at the pair's smaller trim
                # offset: the narrower tile's leading columns hold junk that
                # gets exp'd but is never read (its ctx chunks are skipped)
                w0 = min(w0s)
                nc.scalar.activation(
                    pexp_t[:, :, w0:512], stp[:, :, w0:512],
                    mybir.ActivationFunctionType.Exp, scale=EXP_SCALE)
                for j in range(2):
                    sk_t = 2 * skp + j
                    i = sk_t - (n_sk - 4)
                    if i >= 0:
                        # exact-diagonal 128x128 subtile: keep sq >= sk
                        if (sq_t, h) == (NSQ - 1, NH - 1):
                            # last head: Pool's serialized affine chain would
                            # gate the tail — mask on DVE instead
                            with nc.allow_low_precision(reason="bf16 mask"):
                                nc.vector.tensor_tensor(
                                    pexp_t[:, j, P * i:P * (i + 1)],
                                    pexp_t[:, j, P * i:P * (i + 1)],
                                    trimask[:], mybir.AluOpType.mult)
                        else:
                            nc.gpsimd.affine_select(
                                out=pexp_t[:, j, P * i:P * (i + 1)],
                                in_=pexp_t[:, j, P * i:P * (i + 1)],
                                compare_op=mybir.AluOpType.is_ge,
                                fill=fill0, base=0,
                                channel_multiplier=-1, pattern=[[1, P]])
                return pexp_t

            def ctx_mm_list(sq_t, h, pexps):
                """s-major ctx matmuls for head h; returns (emitters, cps)."""
                n_sk = 4 * (sq_t + 1)
                cps = ps_ctx.tile([P, 4, HD + 1], F32, tag="ctxps", name="cps")
                mms = []
                for s in range(4):
                    sk_hi = min(n_sk - 1, n_sk - 4 + s)
                    # ascending sk: the affine-masked diagonal chunk comes
                    # last, so its GPSIMD mask has drained by then
                    for sk_t in range(0, sk_hi + 1):
                        def mm(s=s, sk_t=sk_t, sk_hi=sk_hi):
                            skp, j = divmod(sk_t, 2)
                            nc.tensor.matmul(
                                cps[:, s, :],
                                pexps[skp][:, j, P * s:P * (s + 1)],
                                v_aug[:, sk_t, h, :],
                                start=(sk_t == 0), stop=(sk_t == sk_hi))
                        mms.append(mm)
                return mms, cps

            def emit_norm(h, cps, pair):
                """DVE reciprocal + broadcast multiply into the pair tile."""
                rcps = csqp.tile([P, 4, 1], F32, tag="rcp", name="rcps")
                with nc.allow_low_precision(reason="softmax recip + bf16 ctx"):
                    nc.vector.reciprocal(rcps[:, :, 0], cps[:, :, HD])
                    b0, b1 = broadcast_tensor_aps(cps[:, :, 0:HD], rcps[:, :, 0:1])
                    nc.vector.tensor_tensor(pair[:, :, h % 2, :], b0, b1,
                                            mybir.AluOpType.mult)

            # stream state
            oproj_accs = {}      # (es, sqs) -> staged c0..c2 sums
            prev = None          # (sq_t, h, pexps, qTs) waiting for ctx
            pair_cur = None      # [P, 4, 2, HD] tile collecting a head pair
            ctxT_cur = None
            ctxT8_cur = None
            ctxT_by_block = {}
            ctxT8_by_block = {}

            seq = [(b, h) for b in range(NSQ) for h in range(NH)]
            for b, h in seq:
                sq_t = b
                n_skp = 2 * (sq_t + 1)
                if h == 0:
                    # block boundary: issue next-block input DMAs first so
                    # they stream during the boundary drain, then emit the
                    # kv/qt fillers this block's first score matmul needs
                    kvb_n = xb_n = None
                    if b + 1 < NSQ:
                        kvb_n = blkp.tile([P, 2, NDO, 512], FP8, tag="blk",
                                          name="kvb")
                        nc.sync.dma_start(
                            kvb_n[:],
                            kvT_r[:, :, :, 512 * (b + 1):512 * (b + 2)])
                        xb_n = blkp.tile([P, 2, NDO, 512], FP8, tag="blk",
                                         name="xb")
                        nc.sync.dma_start(
                            xb_n[:], xT_r[:, :, :, 512 * (b + 1):512 * (b + 2)])
                    drain(len(fillers))
                    if b not in qTs_by_block:
                        qTs_by_block[b] = queue_qt(b)
                        drain(len(fillers))
                    ctxT_cur = ctp.tile([P, NSQ, 4, P], BF16, tag="ctxT",
                                        name="ctxT")
                    ctxT8_cur = ct8p.tile([P, NSQ, 2, 4, P], FP8, tag="ctxT8",
                                          name="ctxT8")
                    ctxT_by_block[b] = ctxT_cur
                    ctxT8_by_block[b] = ctxT8_cur
                    if b + 1 < NSQ:
                        queue_kv_block(b + 1, kvb=kvb_n)
                        qTs_by_block[b + 1] = queue_qt(b + 1, xb=xb_n)
                if (b, h) == (0, 4):
                    # wo streams behind block 1's inputs; first needed by
                    # oproj(b0) two blocks later
                    nc.sync.dma_start(
                        wo[:], wo_d.rearrange("(c p) t e -> p c t e", p=P))
                    nc.sync.dma_start(
                        wobf[:], wobf_d.rearrange("(c p) e -> p c e", p=P))
                qTs = qTs_by_block[b]
                force_late((b, h // 2))
                if h == 1:
                    # ctx(b, 0) interleaves into this head's stages: its v
                    # chunks must be fully emitted first
                    if b == 0:
                        while v0_left[0] > 0:
                            drain(1)
                    force_late(("v", b))
                    if b >= 1:
                        # prev block done and its last pair transpose has
                        # landed: split to fp8 inline (Pool+DVE, waits
                        # already satisfied) and queue its output projection
                        pctxT = ctxT_by_block.pop(b - 1)
                        pctxT8 = ctxT8_by_block.pop(b - 1)
                        for c in range(NSQ):
                            run_ctx_split(pctxT, pctxT8, c)
                        queue_oproj(pctxT8, 512 * (b - 1))

                # ctx work for the previous (block, head), interleaved into
                # this head's score stages
                if prev is not None:
                    p_sqt, p_h, p_pexps, _, p_ctxT, p_ctxT8 = prev
                    cmms, cps = ctx_mm_list(p_sqt, p_h, p_pexps)
                else:
                    cmms, cps = [], None
                per_stage = (len(cmms) + n_skp - 1) // n_skp if cmms else 0

                # last head of the stream: ascending stages, so its ctx
                # (ascending sk) consumes exps in production order and the
                # tail doesn't HOL-wait on the final exp
                if (b, h) == (NSQ - 1, NH - 1):
                    skp_order = list(range(n_skp))
                else:
                    skp_order = list(range(n_skp - 1, -1, -1))
                if b == NSQ - 1 and h == 7:
                    # most of the final block's oproj contraction (c0..c2)
                    # runs in the ACT-bound window once heads 0-5 are
                    # transposed; only c3 + a staged re-add remain after the
                    # last exp. c0/c1 splits first (their transposes landed).
                    queue_ctx_split(ctxT_cur, ctxT8_cur, 0)
                    queue_ctx_split(ctxT_cur, ctxT8_cur, 1)
                    queue_oproj_firsthalf(ctxT_cur, ctxT8_cur, oproj_accs)
                pexps = {}
                for si, skp in enumerate(skp_order):
                    pexps[skp] = emit_stage(sq_t, h, skp, qTs)
                    half = (per_stage + 1) // 2
                    for _ in range(half):
                        if cmms:
                            cmms.pop(0)()
                    drain(1)
                    for _ in range(per_stage - half):
                        if cmms:
                            cmms.pop(0)()
                    drain(DRAIN2[b])
                while cmms:
                    cmms.pop(0)()
                if prev is not None:
                    p_sqt, p_h, p_pexps, _, p_ctxT, p_ctxT8 = prev
                    if p_h % 2 == 0:
                        pair_cur = pairp.tile([P, 4, 2, HD], BF16, tag="pair",
                                              name="pair")
                    emit_norm(p_h, cps, pair_cur)
                    if p_h % 2 == 1 and not (p_sqt == NSQ - 1 and p_h == 7):
                        # pair complete: transpose [sq, hd2] -> [hd2, sq] on
                        # the DMA XBAR, straight into the ctxT tile
                        nc.sync.dma_start_transpose(
                            p_ctxT[:, p_h // 2, :, :], pair_cur[:])

                prev = (sq_t, h, pexps, qTs, ctxT_cur, ctxT8_cur)

            # ---- tail: ctx for the last head, then final oproj
            p_sqt, p_h, p_pexps, _, p_ctxT, p_ctxT8 = prev
            cmms, cps = ctx_mm_list(p_sqt, p_h, p_pexps)
            ci = 0
            while cmms:
                cmms.pop(0)()
                ci += 1
                if ci % 2 == 0:
                    drain(1)
            emit_norm(p_h, cps, pair_cur)
            drain(len(fillers) + len(late))
            # final pair (heads 6,7) PE-transposed: 4 [128,128] transposes +
            # one DVE copy — keeps the tail free of DMA-transpose latency
            tps2 = ps_ctx.tile([P, 4, P], BF16, tag="ctxps", name="tps2")
            for s in range(4):
                nc.tensor.transpose(tps2[:, s, :],
                                    pair_cur[:, s, :, :].rearrange(
                                        "p a b -> p (a b)"),
                                    ident[:])
            nc.vector.tensor_copy(p_ctxT[:, 3, :, :], tps2[:])
            queue_oproj_final(p_ctxT, 512 * p_sqt, oproj_accs)
            drain(len(fillers) + len(late))

    _split_multi_waits(nc)
    return nc


_CACHED = None


def _get_nc():
    global _CACHED
    if _CACHED is None:
        _CACHED = _build()
    return _CACHED


def _make_in_maps(x, kv, Wq, Wk, Wv, Wo):
    import ml_dtypes
    bf = ml_dtypes.bfloat16
    f8 = ml_dtypes.float8_e4m3

    def split8(a, scale=1.0):
        a32 = np.asarray(a, np.float32).astype(bf).astype(np.float32) * scale
        hi = a32.astype(f8)
        lo = (a32 - hi.astype(np.float32)).astype(f8)
        return hi, lo

    def pack_hl_act(hi, lo):
        # [2*D, S] slot-outer so block DMAs stay 3-dim balanceable
        return np.ascontiguousarray(
            np.stack([hi, lo], axis=0).reshape(2 * hi.shape[0], -1))

    def pack_hl(hi, lo, axis=1):
        return np.ascontiguousarray(np.stack([hi, lo], axis=axis))

    x = np.asarray(x, np.float32)
    kv = np.asarray(kv, np.float32)
    WqT = np.asarray(Wq, np.float32).T / np.sqrt(np.float32(HD))
    WkT = np.asarray(Wk, np.float32).T
    WvT = np.asarray(Wv, np.float32).T
    WoT = np.asarray(Wo, np.float32).T
    ident = np.eye(P, dtype=bf)
    in_maps = []
    for c in range(8):
        b, g = c // 2, c % 2
        e0 = g * EG
        xh, xl = split8(x[b].T)
        kvh, kvl = split8(kv[b].T)
        wqh, wql = split8(WqT[:, e0:e0 + EG], WSCALE)
        wkh, wkl = split8(WkT[:, e0:e0 + EG], WSCALE)
        wvh, wvl = split8(WvT[:, e0:e0 + EG], WSCALE)
        woh, wol = split8(WoT[e0:e0 + EG, :], WSCALE)
        wobf = np.ascontiguousarray(
            (WoT[e0 + 2 * P:e0 + 4 * P, :] * WSCALE).astype(bf))
        in_maps.append({
            "xT8": pack_hl_act(xh, xl),
            "kvT8": pack_hl_act(kvh, kvl),
            "wq8": pack_hl(wqh, wql),
            "wk8": pack_hl(wkh, wkl),
            "wv8": pack_hl(wvh, wvl),
            "wo8": pack_hl(woh, wol),
            "wobf": wobf,
            "ident": ident,
        })
    return in_maps


_RUNNER = None


def _get_runner():
    """Persistent jitted 8-core runner (mirrors bass2jax.run_bass_via_pjrt's
    multi-core path, but reusable across kernel() calls so the NEFF is
    compiled once)."""
    global _RUNNER
    if _RUNNER is not None:
        return _RUNNER
    import jax
    from jax.sharding import Mesh, PartitionSpec, NamedSharding
    from jax.experimental.shard_map import shard_map
    from concourse import bass2jax

    nc = _get_nc()
    bass2jax.install_neuronx_cc_hook()
    pname = nc.partition_id_tensor.name if nc.partition_id_tensor else None
    in_names, out_names, out_avals, zero_outs = [], [], [], []
    for alloc in nc.m.functions[0].allocations:
        if not isinstance(alloc, mybir.MemoryLocationSet):
            continue
        name = alloc.memorylocations[0].name
        if alloc.kind == "ExternalInput":
            if name != pname:
                in_names.append(name)
        elif alloc.kind == "ExternalOutput":
            out_names.append(name)
            shape = tuple(alloc.tensor_shape)
            dtype = mybir.dt.np(alloc.dtype)
            out_avals.append(jax.core.ShapedArray(shape, dtype))
            zero_outs.append(np.zeros(shape, dtype))
    n_params = len(in_names)
    all_in = in_names + out_names + ([pname] if pname else [])

    def _body(*args):
        operands = list(args)
        if pname:
            operands.append(bass2jax.partition_id_tensor())
        outs = bass2jax._bass_exec_p.bind(
            *operands, out_avals=tuple(out_avals), in_names=tuple(all_in),
            out_names=tuple(out_names), lowering_input_output_aliases=(),
            sim_require_finite=True, sim_require_nnan=True, nc=nc)
        return tuple(outs)

    n_cores = 8
    try:
        devices = jax.devices("axon")[:n_cores]
    except Exception:
        devices = jax.devices()[:n_cores]
    assert len(devices) == n_cores, f"need {n_cores} neuron cores, have {devices}"
    mesh = Mesh(np.asarray(devices), ("core",))
    donate = tuple(range(n_params, n_params + len(out_names)))
    fn = jax.jit(shard_map(
        _body, mesh=mesh,
        in_specs=(PartitionSpec("core"),) * (n_params + len(out_names)),
        out_specs=(PartitionSpec("core"),) * len(out_names), check_rep=False),
        donate_argnums=donate, keep_unused=True)
    sh = NamedSharding(mesh, PartitionSpec("core"))
    _RUNNER = (fn, in_names, out_names, zero_outs, sh, n_cores)
    return _RUNNER


def _run(x, kv, Wq, Wk, Wv, Wo):
    import jax

    in_maps = _make_in_maps(x, kv, Wq, Wk, Wv, Wo)
    fn, in_names, out_names, zero_outs, sh, n_cores = _get_runner()
    concat_in = [
        np.concatenate([m[name] for m in in_maps], axis=0) for name in in_names
    ]
    concat_z = [np.concatenate([z] * n_cores, axis=0) for z in zero_outs]
    args = [jax.device_put(a, sh) for a in concat_in + concat_z]
    outs = fn(*args)
    per_core = np.asarray(outs[out_names.index("out")]).reshape(
        n_cores, S, D).astype(np.float32)
    out = np.empty((B, S, D), np.float32)
    unscale = np.float32(1.0 / WSCALE)  # undo the Wo fp8 pre-scale
    for b in range(B):
        out[b] = (per_core[2 * b] + per_core[2 * b + 1]) * unscale
    return out


def kernel(x, kv, mask, Wq, Wk, Wv, Wo):
    return _run(x, kv, Wq, Wk, Wv, Wo)
